# revision 1
# baseline (speedup 1.0000x reference)
"""DeepSeek-MoE SwiGLU expert layer on 8 TRN2 NeuronCores (expert parallelism).

Strategy (hardcoded for T=4096, D=1024, DFF=1408, E=8, K=2, 8 cores):
  - Expert parallelism: core e holds expert e's (Wg, Wu, Wd).
  - Dispatch happens at input-sharding time on the host: for each expert,
    gather the tokens routed to it (deduped via the combine matrix), pad to
    capacity C, and ship X^T [D, C] to that core.  Shipping X transposed
    makes every matmul operand on-device natural-layout (contraction dim =
    partition dim), so the kernel needs zero transposes.
  - Per core:  HT = silu(Wg^T @ XT) * (Wu^T @ XT)   [DFF, C]
               YT = Wd^T @ HT                        [D, C]
    fp32r matmuls (full PE rate at N>=256, ~1e-4 relative error), fp32 PSUM
    accumulation.
  - Combine on host: out[idx_e] += (YT[:, :cnt]).T * combine_weight.
"""

import numpy as np
from contextlib import ExitStack

import concourse.bass as bass
import concourse.tile as tile
from concourse import bacc, mybir
from concourse import bass_utils

T, D, DFF, E = 4096, 1024, 1408, 8
N_CORES = 8
P = 128
CT = 512  # matmul moving-operand width (one PSUM bank of fp32)

_cache = {}


def _c_tiles(C):
    tiles = []
    off = 0
    while off < C:
        w = min(CT, C - off)
        tiles.append((off, w))
        off += w
    return tiles


def _emit_body(nc, pools, aps, C):
    DT = mybir.dt.float32r
    f32 = mybir.dt.float32
    KD = D // P    # 8 k-tiles over D
    KF = DFF // P  # 11 k-tiles over DFF
    ctiles = _c_tiles(C)
    xp, hp, wp, pp, sp, op = pools
    xt, wg, wu, wd, yt = aps
    Silu = mybir.ActivationFunctionType.Silu

    def load_w1(f):
        wg_sl = wp.tile([P, KD, P], DT, tag="wg", name=f"wg_sl{f}")
        nc.sync.dma_start(
            out=wg_sl[:],
            in_=wg[:, f * P:(f + 1) * P].rearrange("(k p) m -> p k m", p=P))
        wu_sl = wp.tile([P, KD, P], DT, tag="wu", name=f"wu_sl{f}")
        nc.sync.dma_start(
            out=wu_sl[:],
            in_=wu[:, f * P:(f + 1) * P].rearrange("(k p) m -> p k m", p=P))
        return wg_sl, wu_sl

    # Issue the first f-tile's weight DMAs before the (larger) XT chunk DMAs
    # so the first matmul group isn't queued behind all of XT.
    w1_0 = load_w1(0)

    xt3 = xt.rearrange("(k p) c -> p k c", p=P)
    x_sb = {}
    for i, (c0, cw) in enumerate(ctiles):
        for k in range(KD):
            x_sb[i, k] = xp.tile([P, cw], DT, tag=f"x{i}k{k}",
                                 name=f"x_sb{i}_{k}")
            nc.sync.dma_start(out=x_sb[i, k][:], in_=xt3[:, k, c0:c0 + cw])

    h_sb = {}
    for i, (c0, cw) in enumerate(ctiles):
        h_sb[i] = hp.tile([P, KF, cw], DT, tag=f"h{i}", name=f"h_sb{i}")

    # stage 1: HT[f, c] = silu(Wg^T XT) * (Wu^T XT), transposed space
    for f in range(KF):
        wg_sl, wu_sl = w1_0 if f == 0 else load_w1(f)
        for i, (c0, cw) in enumerate(ctiles):
            ps_g = pp.tile([P, CT], f32, tag="psg")
            ps_u = pp.tile([P, CT], f32, tag="psu")
            for k in range(KD):
                nc.tensor.matmul(ps_g[:, :cw], lhsT=wg_sl[:, k, :],
                                 rhs=x_sb[i, k][:],
                                 start=(k == 0), stop=(k == KD - 1))
            for k in range(KD):
                nc.tensor.matmul(ps_u[:, :cw], lhsT=wu_sl[:, k, :],
                                 rhs=x_sb[i, k][:],
                                 start=(k == 0), stop=(k == KD - 1))
            sg = sp.tile([P, CT], f32)
            nc.scalar.activation(sg[:, :cw], ps_g[:, :cw], Silu)
            nc.vector.tensor_mul(h_sb[i][:, f, :], sg[:, :cw], ps_u[:, :cw])

    # stage 2: YT[dout, c] = Wd^T @ HT
    for do in range(KD):
        wd_sl = wp.tile([P, KF, P], DT, tag="wd")
        nc.sync.dma_start(
            out=wd_sl[:],
            in_=wd[:, do * P:(do + 1) * P].rearrange("(k p) m -> p k m", p=P))
        for i, (c0, cw) in enumerate(ctiles):
            ps_y = pp.tile([P, CT], f32, tag="psy")
            for k in range(KF):
                nc.tensor.matmul(ps_y[:, :cw], lhsT=wd_sl[:, k, :],
                                 rhs=h_sb[i][:, k, :],
                                 start=(k == 0), stop=(k == KF - 1))
            y_sb = op.tile([P, CT], f32)
            nc.vector.tensor_copy(y_sb[:, :cw], ps_y[:, :cw])
            nc.sync.dma_start(out=yt[do * P:(do + 1) * P, c0:c0 + cw],
                              in_=y_sb[:, :cw])


def _declare(nc, C):
    DT = mybir.dt.float32r
    f32 = mybir.dt.float32
    xt = nc.dram_tensor("xt", [D, C], DT, kind="ExternalInput").ap()
    wg = nc.dram_tensor("wg", [D, DFF], DT, kind="ExternalInput").ap()
    wu = nc.dram_tensor("wu", [D, DFF], DT, kind="ExternalInput").ap()
    wd = nc.dram_tensor("wd", [DFF, D], DT, kind="ExternalInput").ap()
    yt = nc.dram_tensor("yt", [D, C], f32, kind="ExternalOutput").ap()
    return (xt, wg, wu, wd, yt)


def _pools(tc, ctx):
    xp = ctx.enter_context(tc.tile_pool(name="xt_p", bufs=1))
    hp = ctx.enter_context(tc.tile_pool(name="ht_p", bufs=1))
    wp = ctx.enter_context(tc.tile_pool(name="w_p", bufs=3))
    pp = ctx.enter_context(tc.tile_pool(name="ps_p", bufs=2, space="PSUM"))
    sp = ctx.enter_context(tc.tile_pool(name="sg_p", bufs=4))
    op = ctx.enter_context(tc.tile_pool(name="y_p", bufs=4))
    return (xp, hp, wp, pp, sp, op)


def _build(C):
    key = ("plain", C)
    if key in _cache:
        return _cache[key]
    nc = bacc.Bacc("TRN2", target_bir_lowering=False, debug=False,
                   num_devices=N_CORES)
    aps = _declare(nc, C)
    with tile.TileContext(nc) as tc, ExitStack() as ctx:
        pools = _pools(tc, ctx)
        _emit_body(nc, pools, aps, C)
    nc.compile()
    _cache[key] = nc
    return nc


def _build_loop(C):
    """Benchmark variant: repeat the body niter times (runtime input)."""
    key = ("loop", C)
    if key in _cache:
        return _cache[key]
    nc = bacc.Bacc("TRN2", target_bir_lowering=False, debug=False,
                   num_devices=N_CORES)
    aps = _declare(nc, C)
    n_ap = nc.dram_tensor("niter", [1, 1], mybir.dt.uint32,
                          kind="ExternalInput").ap()
    with tile.TileContext(nc) as tc, ExitStack() as ctx:
        cpool = ctx.enter_context(tc.tile_pool(name="c_p", bufs=1))
        pools = _pools(tc, ctx)
        n_sb = cpool.tile([1, 1], mybir.dt.uint32)
        nc.sync.dma_start(out=n_sb[:], in_=n_ap[:])
        with tc.tile_critical():
            tmp = nc.alloc_registers("niter_regs")
            nc.regs_load(tmp, n_sb[0:1, 0:1])
            n_val = nc.snap(tmp, donate=True, min_val=0, max_val=1 << 20)
        with tc.For_i(0, n_val, 1, hint_engines=(mybir.EngineType.PE,)):
            _emit_body(nc, pools, aps, C)
    nc.compile()
    _cache[key] = nc
    return nc


def _dispatch(x, topk_weights, topk_indices, num_experts):
    """Host-side routing: combine matrix + per-expert token index lists."""
    T_, _ = x.shape
    E_ = int(num_experts)
    ti = np.asarray(topk_indices).astype(np.int64)
    tw = np.asarray(topk_weights).astype(np.float32)
    combine = np.zeros((T_, E_), np.float32)
    np.add.at(combine, (np.arange(T_)[:, None], ti), tw)
    idxs = [np.nonzero(combine[:, e])[0] for e in range(E_)]
    return combine, idxs


def _capacity(idxs):
    maxc = max((len(i) for i in idxs), default=0)
    return max(CT, ((maxc + P - 1) // P) * P)


def _in_maps(x, Wg, Wu, Wd, idxs, C):
    maps = []
    D_ = x.shape[1]
    for e in range(len(idxs)):
        xt_e = np.zeros((D_, C), np.float32)
        n = len(idxs[e])
        if n:
            xt_e[:, :n] = x[idxs[e]].T
        maps.append({
            "xt": xt_e,
            "wg": np.ascontiguousarray(Wg[e], np.float32),
            "wu": np.ascontiguousarray(Wu[e], np.float32),
            "wd": np.ascontiguousarray(Wd[e], np.float32),
        })
    return maps


def kernel(x, Wg, Wu, Wd, topk_weights, topk_indices, num_experts):
    x = np.asarray(x, np.float32)
    Wg = np.asarray(Wg, np.float32)
    Wu = np.asarray(Wu, np.float32)
    Wd = np.asarray(Wd, np.float32)
    T_, D_ = x.shape

    combine, idxs = _dispatch(x, topk_weights, topk_indices, num_experts)
    C = _capacity(idxs)

    nc = _build(C)
    res = bass_utils.run_bass_kernel_spmd(nc, _in_maps(x, Wg, Wu, Wd, idxs, C),
                                          list(range(N_CORES)))

    out = np.zeros((T_, D_), np.float32)
    for e in range(len(idxs)):
        n = len(idxs[e])
        if n:
            ye = res.results[e]["yt"][:, :n].T
            out[idxs[e]] += ye * combine[idxs[e], e][:, None]
    return out



# revision 13
# speedup vs baseline: 2.8279x; 2.8279x over previous
"""DeepSeek-MoE SwiGLU expert layer on 8 TRN2 NeuronCores (expert parallelism).

Strategy (hardcoded for T=4096, D=1024, DFF=1408, E=8, K=2, 8 cores):
  - Expert parallelism: core e holds expert e's (Wg, Wu, Wd).
  - Dispatch happens at input-sharding time on the host: for each expert,
    gather the tokens routed to it (deduped via the combine matrix), pad to
    capacity C, and ship X^T to that core.  Shipping X transposed makes every
    matmul operand on-device natural-layout (contraction dim = partition
    dim), so the kernel needs zero transposes.
  - All matmul operands are bf16 (absmax rel err ~4e-3, well inside 2e-2);
    PSUM accumulation and the output stay fp32.
  - Weights are pre-tiled on the host so every DMA lands with >=512B
    contiguous per partition line (full DMA-engine rate, few descriptors):
      wgu[f, k, p, s, m] = (Wg if s==0 else Wu)[k*128+p, f*128+m]
      wd[do, p, k*128+m] = Wd[k*128+p, do*128+m]
      xt[k, p, c]        = x[token c, k*128+p]
  - Per core:  HT = silu(Wg^T @ XT) * (Wu^T @ XT)   [DFF, C]
               YT = Wd^T @ HT                        [D, C]
    Column blocks of 512 are processed outermost so stage 2 of block 0
    overlaps stage 1 of block 1; weights stay resident in SBUF.
  - DMA issue is spread across engine queues (weights on SP, x and output
    drains split between Activation and DVE) because each queue serializes
    its DMAs; outputs are DMA'd straight from PSUM.
  - Combine on host: out[idx_e] += (YT[:, :cnt]).T * combine_weight.
"""

import numpy as np
import ml_dtypes
from contextlib import ExitStack

import concourse.bass as bass
import concourse.tile as tile
from concourse import bacc, mybir
from concourse import bass_utils

T, D, DFF, E = 4096, 1024, 1408, 8
N_CORES = 8
P = 128
KD = D // P    # 8 k-tiles over D
KF = DFF // P  # 11 k-tiles over DFF
CT = 512       # matmul moving-operand width (one PSUM bank of fp32)

BF16 = ml_dtypes.bfloat16

_cache = {}


def _c_tiles(C):
    tiles = []
    off = 0
    while off < C:
        w = min(CT, C - off)
        tiles.append((off, w))
        off += w
    return tiles


def _emit_body(nc, pools, aps, C):
    bf = mybir.dt.bfloat16
    f32 = mybir.dt.float32
    ctiles = _c_tiles(C)
    NI = len(ctiles)
    xp, hp, wp, pp, sp, op = pools
    xt, wgu, wd, yt = aps
    Silu = mybir.ActivationFunctionType.Silu
    KH = KD // 2  # k-half for x loads

    # --- f=0 weights in two k-half pieces (fast start), on SP ---
    wgu0 = []
    for h in range(2):
        t = wp.tile([P, KD // 2, 2, P], bf, tag=f"wgu0{h}", name=f"wgu0{h}")
        nc.sync.dma_start(
            out=t[:],
            in_=wgu[0, h * (KD // 2):(h + 1) * (KD // 2)].rearrange(
                "k p s m -> p k s m"))
        wgu0.append(t)

    # --- x, one [P, KD/2, cw] DMA per (ctile, k-half) on ACT.  Only ctile 0
    #     is loaded upfront; later ctiles are emitted mid-f-loop so they
    #     don't delay the first silus on ACT's serial queue. ---
    x_sb = {}

    def emit_x(i):
        c0, cw = ctiles[i]
        for h in range(2):
            t = xp.tile([P, KH, cw], bf, tag=f"x{i}h{h}", name=f"x{i}h{h}")
            nc.scalar.dma_start(
                out=t[:],
                in_=xt[h * KH:(h + 1) * KH, :, c0:c0 + cw].rearrange(
                    "k p c -> p k c"))
            x_sb[i, h] = t

    emit_x(0)

    def x_view(i, k):
        return x_sb[i, k // KH][:, k % KH, :]

    # --- remaining weights, all resident, streamed on SP ---
    wgu_sl = {}
    for f in range(1, KF):
        t = wp.tile([P, KD, 2, P], bf, tag=f"wgu{f}", name=f"wgu{f}")
        nc.sync.dma_start(out=t[:], in_=wgu[f].rearrange("k p s m -> p k s m"))
        wgu_sl[f] = t
    wd_sl = {}
    for do in range(KD):
        t = wp.tile([P, KF * P], bf, tag=f"wd{do}", name=f"wd{do}")
        nc.sync.dma_start(out=t[:], in_=wd[do])
        wd_sl[do] = t

    def w1_view(f, k, s):
        if f == 0:
            return wgu0[k // (KD // 2)][:, k % (KD // 2), s, :]
        return wgu_sl[f][:, k, s, :]

    h_sb = {}
    for i, (c0, cw) in enumerate(ctiles):
        # --- stage 1: HT[f, c] = silu(Wg^T XT) * (Wu^T XT) for this ctile ---
        h_sb[i] = hp.tile([P, KF, cw], bf, tag=f"h{i}", name=f"h_sb{i}")
        for f in range(KF):
            if i + 1 < NI and f == 3:
                emit_x(i + 1)
            ps_g = pp.tile([P, CT], f32, tag="psg")
            ps_u = pp.tile([P, CT], f32, tag="psu")
            ps = {0: ps_g, 1: ps_u}
            if i == 0 and f == 0:
                # consume k halves in arrival order so the PE stays fed
                # while the second x/weight halves are still landing
                order = [(0, range(KH)), (1, range(KH)),
                         (0, range(KH, KD)), (1, range(KH, KD))]
            else:
                order = [(0, range(KD)), (1, range(KD))]
            for s, ks in order:
                for k in ks:
                    nc.tensor.matmul(ps[s][:, :cw], lhsT=w1_view(f, k, s),
                                     rhs=x_view(i, k),
                                     start=(k == 0), stop=(k == KD - 1))
            sg = sp.tile([P, CT], f32)
            nc.scalar.activation(sg[:, :cw], ps_g[:, :cw], Silu)
            nc.vector.tensor_mul(h_sb[i][:, f, :], sg[:, :cw], ps_u[:, :cw])

        # --- stage 2: YT[do, c] = Wd^T @ HT; DVE drains PSUM -> bf16 SBUF ---
        for do in range(KD):
            ps_y = pp.tile([P, CT], f32, tag="psy")
            for k in range(KF):
                nc.tensor.matmul(ps_y[:, :cw], lhsT=wd_sl[do][:, k * P:(k + 1) * P],
                                 rhs=h_sb[i][:, k, :],
                                 start=(k == 0), stop=(k == KF - 1))
            y_sb = op.tile([P, CT], bf)
            last = (i == NI - 1) and (do == KD - 1)
            row = yt[do * P:(do + 1) * P]
            if last and cw > 1:
                # pipeline the final drain: two half copies feeding two
                # parallel DMA queues, for the shortest possible tail
                half = cw // 2
                nc.vector.tensor_copy(y_sb[:, :half], ps_y[:, :half])
                nc.sync.dma_start(out=row[:, c0:c0 + half],
                                  in_=y_sb[:, :half])
                nc.vector.tensor_copy(y_sb[:, half:cw], ps_y[:, half:cw])
                nc.scalar.dma_start(out=row[:, c0 + half:c0 + cw],
                                    in_=y_sb[:, half:cw])
            else:
                nc.vector.tensor_copy(y_sb[:, :cw], ps_y[:, :cw])
                # ctile 0 drains on ACT (SP still streaming weights);
                # later ctiles drain on SP (weights done by then).
                eng = nc.scalar if i == 0 else nc.sync
                eng.dma_start(out=row[:, c0:c0 + cw], in_=y_sb[:, :cw])


def _declare(nc, C):
    bf = mybir.dt.bfloat16
    xt = nc.dram_tensor("xt", [KD, P, C], bf, kind="ExternalInput").ap()
    wgu = nc.dram_tensor("wgu", [KF, KD, P, 2, P], bf,
                         kind="ExternalInput").ap()
    wd = nc.dram_tensor("wd", [KD, P, KF * P], bf, kind="ExternalInput").ap()
    yt = nc.dram_tensor("yt", [D, C], bf, kind="ExternalOutput").ap()
    return (xt, wgu, wd, yt)


def _pools(tc, ctx):
    xp = ctx.enter_context(tc.tile_pool(name="xt_p", bufs=1))
    hp = ctx.enter_context(tc.tile_pool(name="ht_p", bufs=1))
    wp = ctx.enter_context(tc.tile_pool(name="w_p", bufs=1))
    pp = ctx.enter_context(tc.tile_pool(name="ps_p", bufs=2, space="PSUM"))
    sp = ctx.enter_context(tc.tile_pool(name="sg_p", bufs=2))
    op = ctx.enter_context(tc.tile_pool(name="y_p", bufs=3))
    return (xp, hp, wp, pp, sp, op)


def _build(C):
    key = ("plain", C)
    if key in _cache:
        return _cache[key]
    nc = bacc.Bacc("TRN2", target_bir_lowering=False, debug=False,
                   num_devices=N_CORES)
    aps = _declare(nc, C)
    with tile.TileContext(nc) as tc, ExitStack() as ctx:
        pools = _pools(tc, ctx)
        _emit_body(nc, pools, aps, C)
    nc.compile()
    _cache[key] = nc
    return nc


def _build_loop(C):
    """Benchmark variant: repeat the body niter times (runtime input)."""
    key = ("loop", C)
    if key in _cache:
        return _cache[key]
    nc = bacc.Bacc("TRN2", target_bir_lowering=False, debug=False,
                   num_devices=N_CORES)
    aps = _declare(nc, C)
    n_ap = nc.dram_tensor("niter", [1, 1], mybir.dt.uint32,
                          kind="ExternalInput").ap()
    with tile.TileContext(nc) as tc, ExitStack() as ctx:
        cpool = ctx.enter_context(tc.tile_pool(name="c_p", bufs=1))
        pools = _pools(tc, ctx)
        n_sb = cpool.tile([1, 1], mybir.dt.uint32)
        nc.sync.dma_start(out=n_sb[:], in_=n_ap[:])
        with tc.tile_critical():
            tmp = nc.alloc_registers("niter_regs")
            nc.regs_load(tmp, n_sb[0:1, 0:1])
            n_val = nc.snap(tmp, donate=True, min_val=0, max_val=1 << 20)
        with tc.For_i(0, n_val, 1, hint_engines=(mybir.EngineType.PE,)):
            _emit_body(nc, pools, aps, C)
    nc.compile()
    _cache[key] = nc
    return nc


def _dispatch(x, topk_weights, topk_indices, num_experts):
    """Host-side routing: combine matrix + per-expert token index lists."""
    T_, _ = x.shape
    E_ = int(num_experts)
    ti = np.asarray(topk_indices).astype(np.int64)
    tw = np.asarray(topk_weights).astype(np.float32)
    combine = np.zeros((T_, E_), np.float32)
    np.add.at(combine, (np.arange(T_)[:, None], ti), tw)
    idxs = [np.nonzero(combine[:, e])[0] for e in range(E_)]
    return combine, idxs


def _capacity(idxs):
    maxc = max((len(i) for i in idxs), default=0)
    return max(64, ((maxc + 31) // 32) * 32)


def _in_maps(x, Wg, Wu, Wd, idxs, C):
    maps = []
    D_ = x.shape[1]
    for e in range(len(idxs)):
        xt_e = np.zeros((D_, C), BF16)
        n = len(idxs[e])
        if n:
            xt_e[:, :n] = x[idxs[e]].astype(BF16).T
        wg4 = np.asarray(Wg[e], BF16).reshape(KD, P, KF, P)
        wu4 = np.asarray(Wu[e], BF16).reshape(KD, P, KF, P)
        wgu = np.ascontiguousarray(
            np.stack([wg4, wu4], axis=3).transpose(2, 0, 1, 3, 4))
        wd4 = np.asarray(Wd[e], BF16).reshape(KF, P, KD, P)
        wdt = np.ascontiguousarray(
            wd4.transpose(2, 1, 0, 3).reshape(KD, P, KF * P))
        maps.append({
            "xt": xt_e.reshape(KD, P, C),
            "wgu": wgu,
            "wd": wdt,
        })
    return maps


def kernel(x, Wg, Wu, Wd, topk_weights, topk_indices, num_experts):
    x = np.asarray(x, np.float32)
    Wg = np.asarray(Wg, np.float32)
    Wu = np.asarray(Wu, np.float32)
    Wd = np.asarray(Wd, np.float32)
    T_, D_ = x.shape

    combine, idxs = _dispatch(x, topk_weights, topk_indices, num_experts)
    C = _capacity(idxs)

    nc = _build(C)
    res = bass_utils.run_bass_kernel_spmd(nc, _in_maps(x, Wg, Wu, Wd, idxs, C),
                                          list(range(N_CORES)))

    out = np.zeros((T_, D_), np.float32)
    for e in range(len(idxs)):
        n = len(idxs[e])
        if n:
            ye = res.results[e]["yt"][:, :n].T.astype(np.float32)
            out[idxs[e]] += ye * combine[idxs[e], e][:, None]
    return out
